# revision 1
# baseline (speedup 1.0000x reference)
"""Depthwise Conv1d (C=128, K=3, stride=1, pad=1) Trainium2 Bass kernel.

Layout: partitions = channels (C=128 exactly matches SBUF partitions).
Sharding: data-parallel over batch — 32 images / 8 cores = 4 images/core.
Per tile [128, N]:
    ACT : mid = w1 * x_center + bias          (activation Identity, per-partition scale/bias)
    DVE : acc = (x_left  * w0) + mid          (scalar_tensor_tensor)
    DVE : res = (x_right * w2) + acc          (scalar_tensor_tensor)
The kernel is HBM-bandwidth bound (~33.6 MB/core mandatory traffic).
Stores issue on the scalar HWDGE ring so a store waiting on compute never
head-of-line-blocks the next load on the sync ring; the final row tapers
to small tiles so the tail compute+store chain adds little to the DMA span.
"""

import numpy as np

import concourse.bacc as bacc
import concourse.mybir as mybir
import concourse.tile as tile
from concourse import bass_utils

B, C, L, K = 32, 128, 8192, 3
NCORES = 8
BPC = B // NCORES  # images per core

TILE_N = 4096
BUFS_IN = 5
BUFS_MID = 6
BUFS_ACC = 3
SUB_N = 2048

_nc_cache = {}


def _row_widths(bi, tile_n, taper):
    """Tile widths for image row bi (must sum to L)."""
    if taper and bi == BPC - 1:
        # shrink the final tiles so the tail dependency chain
        # (last load -> compute -> last store) is short
        tail = [2048, 1024, 512, 512]
        body = L - sum(tail)
        widths = [tile_n] * (body // tile_n) + tail
        assert sum(widths) == L
        return widths
    return [tile_n] * (L // tile_n)


def _build_nc(
    tile_n=TILE_N,
    bufs_in=BUFS_IN,
    bufs_mid=BUFS_MID,
    bufs_acc=BUFS_ACC,
    store_on_scalar=1,
    taper=0,
    repeat=1,
    memset_on_gpsimd=0,
    gpsimd_every=0,
    const_on_scalar=0,
    sub_n=SUB_N,
):
    f32 = mybir.dt.float32
    nc = bacc.Bacc(
        "TRN2",
        target_bir_lowering=False,
        debug=False,
        enable_asserts=False,
        num_devices=NCORES,
    )
    x = nc.dram_tensor("x", [BPC, C, L], f32, kind="ExternalInput").ap()
    w = nc.dram_tensor("w", [C, K], f32, kind="ExternalInput").ap()
    b = nc.dram_tensor("b", [C, 1], f32, kind="ExternalInput").ap()
    y = nc.dram_tensor("y", [BPC, C, L], f32, kind="ExternalOutput").ap()

    mult = mybir.AluOpType.mult
    add = mybir.AluOpType.add
    ident = mybir.ActivationFunctionType.Identity

    with tile.TileContext(nc) as tc:
        with (
            tc.tile_pool(name="const", bufs=1) as cpool,
            tc.tile_pool(name="work", bufs=1) as pool,
        ):
            wtile = cpool.tile([C, K], f32)
            btile = cpool.tile([C, 1], f32)
            const_eng = nc.scalar if const_on_scalar else nc.sync
            const_eng.dma_start(out=wtile[:, :], in_=w)
            const_eng.dma_start(out=btile[:, :], in_=b)

            store_eng = nc.scalar if store_on_scalar else nc.sync
            memset_eng = nc.gpsimd if memset_on_gpsimd else nc.vector
            it = 0
            for bi in [b for _ in range(repeat) for b in range(BPC)]:
                l0 = 0
                for n in _row_widths(bi, tile_n, taper):
                    # input halo range [l0-1, l0+n+1) clipped to [0, L)
                    lo, hi = l0 - 1, l0 + n + 1
                    src_lo, src_hi = max(lo, 0), min(hi, L)
                    dst = src_lo - lo

                    xin = pool.tile([C, tile_n + 2], f32, tag="xin", bufs=bufs_in)
                    if lo < 0:
                        memset_eng.memset(xin[:, 0:1], 0.0)
                    if hi > L:
                        memset_eng.memset(xin[:, n + 1 : n + 2], 0.0)
                    nc.sync.dma_start(
                        out=xin[:, dst : dst + (src_hi - src_lo)],
                        in_=x[bi, :, src_lo:src_hi],
                    )

                    stt_eng = (
                        nc.gpsimd
                        if gpsimd_every and (it % gpsimd_every == gpsimd_every - 1)
                        else nc.vector
                    )
                    # compute+store in sub_n-wide chunks (loads stay tile_n
                    # wide) to shorten the compute-to-store latency per byte
                    step = sub_n if sub_n and sub_n < n else n
                    for s0 in range(0, n, step):
                        sn = min(step, n - s0)
                        mid = pool.tile([C, step], f32, tag="mid", bufs=bufs_mid)
                        acc = pool.tile([C, step], f32, tag="acc", bufs=bufs_acc)
                        nc.scalar.activation(
                            mid[:, 0:sn],
                            xin[:, s0 + 1 : s0 + sn + 1],
                            ident,
                            bias=btile[:, 0:1],
                            scale=wtile[:, 1:2],
                        )
                        stt_eng.scalar_tensor_tensor(
                            acc[:, 0:sn], xin[:, s0 : s0 + sn],
                            wtile[:, 0:1], mid[:, 0:sn], mult, add
                        )
                        stt_eng.scalar_tensor_tensor(
                            mid[:, 0:sn], xin[:, s0 + 2 : s0 + sn + 2],
                            wtile[:, 2:3], acc[:, 0:sn], mult, add
                        )
                        store_eng.dma_start(
                            out=y[bi, :, l0 + s0 : l0 + s0 + sn], in_=mid[:, 0:sn]
                        )
                    l0 += n
                    it += 1

    nc.compile()
    return nc


def _get_nc(**kw):
    key = tuple(sorted(kw.items()))
    if key not in _nc_cache:
        _nc_cache[key] = _build_nc(**kw)
    return _nc_cache[key]


def kernel_with_results(inputs, weight, bias, trace=False, **build_kw):
    x = np.ascontiguousarray(inputs, dtype=np.float32)
    w = np.ascontiguousarray(weight, dtype=np.float32)
    b = np.ascontiguousarray(bias, dtype=np.float32).reshape(C, 1)
    assert x.shape == (B, C, L), x.shape
    nc = _get_nc(**build_kw)
    in_maps = [
        {"x": x[i * BPC : (i + 1) * BPC], "w": w, "b": b} for i in range(NCORES)
    ]
    res = bass_utils.run_bass_kernel_spmd(
        nc, in_maps, core_ids=list(range(NCORES)), trace=trace
    )
    out = np.concatenate([r["y"] for r in res.results], axis=0)
    return out, res


def kernel(inputs, weight, bias):
    out, _ = kernel_with_results(inputs, weight, bias)
    return out



# revision 27
# speedup vs baseline: 1.1847x; 1.1847x over previous
"""Depthwise Conv1d (C=128, K=3, stride=1, pad=1) Trainium2 Bass kernel.

Layout: partitions = channels (C=128 exactly matches SBUF partitions).
Sharding: data-parallel over batch — 32 images / 8 cores = 4 images/core.
Per tile [128, 2048]:
    ACT : mid = w1 * x_center + bias          (activation Identity, per-partition scale/bias)
    STT : acc = (x_left  * w0) + mid          (scalar_tensor_tensor)
    STT : res = (x_right * w2) + acc          (scalar_tensor_tensor)
The kernel is HBM-bandwidth bound: ~33.6 MB/core mandatory traffic at
~355-405 GB/s (the two NeuronCores on an HBM stack share ~716 GB/s, so
the achieved rate depends on how much the stack-neighbor overlaps).
Design points, each worth measured microseconds on HW:
  - 2048-col load tiles: fine-grained load/store interleave raises the
    achieved HBM rate vs 4096-col tiles (~405 vs ~360 GB/s).
  - Engines are in-order, so a store issued right after its tile's
    compute would make the NEXT tile's ACT wait on this tile's STT2
    (circular ACT->STT->store->ACT serialization). Stores are emitted
    `store_defer` tiles late so the scalar engine never waits.
  - Ramp-up: the first row starts with 256/512/1024-col tiles so the
    first compute starts ~6 us earlier (a full-size first load takes
    ~9 us to complete+signal through a cold DMA queue).
  - Taper: the last row shrinks its final tiles so the tail
    load->compute->store chain is short.
  - Stores ride the scalar HWDGE ring so they never head-of-line-block
    loads on the sync ring; const loads also go on the scalar ring so
    the sync ring's first DMA is input data.
The DVE (2 STT/tile, ~77 us busy) and the DMA stream finish within a
microsecond of each other — both are critical paths. A PE/fp32r
diag-matmul offload path exists behind pe_every>0 but measured slower
(PE runs at mid p-state with unhidden LDWEIGHTS); default keeps it off.
"""

import numpy as np

import concourse.bacc as bacc
import concourse.mybir as mybir
import concourse.tile as tile
from concourse import bass_utils

B, C, L, K = 32, 128, 8192, 3
NCORES = 8
BPC = B // NCORES  # images per core

TILE_N = 2048
BUFS_IN = 10
BUFS_MID = 6
BUFS_ACC = 3
SUB_N = 2048

_nc_cache = {}


def _row_widths(bi, tile_n, taper, ramp=1):
    """Tile widths for image row bi (must sum to L)."""
    if ramp and bi == 0:
        # start with small tiles so the first load completes (and compute
        # starts) as early as possible — the first full-size load takes
        # ~9 us to complete+signal while a 256-col one takes ~2 us
        head = [256, 256, 512, 1024]
        body = L - sum(head)
        widths = head + [tile_n] * (body // tile_n)
        rem = L - sum(widths)
        if rem:
            widths.append(rem)
        assert sum(widths) == L
        return widths
    if taper and bi == BPC - 1:
        # shrink the final tiles so the tail dependency chain
        # (last load -> compute -> last store) is short
        tail = [tile_n, tile_n // 2, tile_n // 4, tile_n // 4]
        body = L - sum(tail)
        widths = [tile_n] * (body // tile_n) + tail
        assert sum(widths) == L
        return widths
    return [tile_n] * (L // tile_n)


def _build_nc(
    tile_n=TILE_N,
    bufs_in=BUFS_IN,
    bufs_mid=BUFS_MID,
    bufs_acc=BUFS_ACC,
    store_on_scalar=1,
    taper=1,
    repeat=1,
    store_defer=2,
    const_on_scalar=1,
    memset_on_pool=1,
    sub_n=SUB_N,
    pe_every=0,
    bufs_psum=2,
    load_ring_alt=0,
    ramp=1,
):
    f32 = mybir.dt.float32
    nc = bacc.Bacc(
        "TRN2",
        target_bir_lowering=False,
        debug=False,
        enable_asserts=False,
        num_devices=NCORES,
    )
    x = nc.dram_tensor("x", [BPC, C, L], f32, kind="ExternalInput").ap()
    w = nc.dram_tensor("w", [C, K], f32, kind="ExternalInput").ap()
    b = nc.dram_tensor("b", [C, 1], f32, kind="ExternalInput").ap()
    y = nc.dram_tensor("y", [BPC, C, L], f32, kind="ExternalOutput").ap()

    f32r = mybir.dt.float32r
    mult = mybir.AluOpType.mult
    add = mybir.AluOpType.add
    ident = mybir.ActivationFunctionType.Identity

    with tile.TileContext(nc) as tc:
        with (
            tc.tile_pool(name="const", bufs=1) as cpool,
            tc.tile_pool(name="work", bufs=1) as pool,
            tc.tile_pool(name="psum", bufs=1, space="PSUM") as ppool,
        ):
            wtile = cpool.tile([C, K], f32)
            btile = cpool.tile([C, 1], f32)
            # consts on the scalar ring: the sync ring's first DMA stays the
            # first input load (0=sync, 2=gpsimd SWDGE measured no better)
            const_eng = {0: nc.sync, 1: nc.scalar, 2: nc.gpsimd}[const_on_scalar]
            const_eng.dma_start(out=wtile[:, :], in_=w)
            const_eng.dma_start(out=btile[:, :], in_=b)

            dk = None
            if pe_every:
                # diag weight matrices for the PE path: D_k = diag(w[:, k]).
                # ones -> affine_select keeps the p==j diagonal -> per-partition
                # scalar multiply by w_k.
                ones = cpool.tile([C, C], f32)
                identm = cpool.tile([C, C], f32)
                dk = cpool.tile([C, K * C], f32)
                nc.gpsimd.memset(ones[:, :], 1.0)
                nc.gpsimd.affine_select(
                    identm[:, :], ones[:, :], pattern=[[-1, C]],
                    compare_op=mybir.AluOpType.is_equal, fill=0.0,
                    base=0, channel_multiplier=1,
                )
                for k in range(K):
                    nc.vector.tensor_scalar_mul(
                        dk[:, k * C : (k + 1) * C].bitcast(f32r),
                        identm[:, :],
                        wtile[:, k : k + 1],
                    )

            store_eng = nc.scalar if store_on_scalar else nc.sync
            memset_eng = nc.gpsimd if memset_on_pool else nc.vector
            pending = []  # deferred stores: (tile, sn, bi, l0+s0)

            def flush_store():
                mid, sn, sbi, sl0 = pending.pop(0)
                store_eng.dma_start(out=y[sbi, :, sl0 : sl0 + sn], in_=mid[:, 0:sn])

            pe_ctr = 0
            tile_ctr = 0
            for bi in [b for _ in range(repeat) for b in range(BPC)]:
                l0 = 0
                for n in _row_widths(bi, tile_n, taper, ramp):
                    # input halo range [l0-1, l0+n+1) clipped to [0, L)
                    lo, hi = l0 - 1, l0 + n + 1
                    src_lo, src_hi = max(lo, 0), min(hi, L)
                    dst = src_lo - lo

                    # PE-path tiles must be produced solely by the DMA (the
                    # BIR verifier requires fp32r matmul inputs to come from
                    # an fp32r-typed producer; memset halos would add another
                    # producer) — so row-edge tiles stay on the DVE path.
                    eligible = (
                        pe_every and lo >= 0 and hi <= L
                        and n % 512 == 0 and (not sub_n or sub_n >= n)
                    )
                    on_pe = False
                    if eligible:
                        on_pe = pe_ctr % pe_every == pe_every - 1
                        pe_ctr += 1

                    nb = bufs_in // 2 if pe_every else bufs_in
                    xin = pool.tile(
                        [C, tile_n + 2], f32r if on_pe else f32,
                        tag="xin_r" if on_pe else "xin",
                        bufs=(bufs_in - nb) if on_pe else nb,
                    )
                    if lo < 0:
                        memset_eng.memset(xin[:, 0:1], 0.0)
                    if hi > L:
                        memset_eng.memset(xin[:, n + 1 : n + 2], 0.0)
                    src = x[bi, :, src_lo:src_hi]
                    load_eng = nc.sync
                    if load_ring_alt and tile_ctr % 2 == 1:
                        load_eng = nc.gpsimd
                    tile_ctr += 1
                    load_eng.dma_start(
                        out=xin[:, dst : dst + (src_hi - src_lo)],
                        in_=src.bitcast(f32r) if on_pe else src,
                    )

                    # compute in sub_n-wide chunks (loads stay tile_n wide)
                    step = sub_n if sub_n and sub_n < n else n
                    for s0 in range(0, n, step):
                        sn = min(step, n - s0)
                        mid = pool.tile([C, step], f32, tag="mid", bufs=bufs_mid)
                        if on_pe:
                            # PE path: out = sum_k D_k @ xin[:, s0+k : ...]
                            # accumulated per 512-col PSUM bank (fp32r runs at
                            # 1 row/cycle for moving dim >= 256), then ACT
                            # drains PSUM -> SBUF folding in the bias.
                            ps = ppool.tile([C, sn], f32, tag="ps", bufs=bufs_psum)
                            for k in range(K):
                                for c0 in range(0, sn, 512):
                                    nc.tensor.matmul(
                                        ps[:, c0 : c0 + 512],
                                        dk[:, k * C : (k + 1) * C].bitcast(f32r),
                                        xin[:, s0 + k + c0 : s0 + k + c0 + 512],
                                        start=(k == 0),
                                        stop=(k == K - 1),
                                    )
                            nc.scalar.activation(
                                mid[:, 0:sn], ps[:, 0:sn], ident,
                                bias=btile[:, 0:1], scale=1.0,
                            )
                        else:
                            acc = pool.tile([C, step], f32, tag="acc", bufs=bufs_acc)
                            nc.scalar.activation(
                                mid[:, 0:sn],
                                xin[:, s0 + 1 : s0 + sn + 1],
                                ident,
                                bias=btile[:, 0:1],
                                scale=wtile[:, 1:2],
                            )
                            nc.vector.scalar_tensor_tensor(
                                acc[:, 0:sn], xin[:, s0 : s0 + sn],
                                wtile[:, 0:1], mid[:, 0:sn], mult, add
                            )
                            nc.vector.scalar_tensor_tensor(
                                mid[:, 0:sn], xin[:, s0 + 2 : s0 + sn + 2],
                                wtile[:, 2:3], acc[:, 0:sn], mult, add
                            )
                        pending.append((mid, sn, bi, l0 + s0))
                        # defer stores mid-stream (so a store waiting on
                        # compute never blocks the next ACT on the in-order
                        # scalar engine); shallower deferral in the taper row
                        # so the final store isn't queued behind stale ones
                        defer = store_defer if bi < BPC - 1 else min(store_defer, 1)
                        while len(pending) > defer:
                            flush_store()
                    l0 += n
            while pending:
                flush_store()

    nc.compile()
    return nc


def _get_nc(**kw):
    key = tuple(sorted(kw.items()))
    if key not in _nc_cache:
        _nc_cache[key] = _build_nc(**kw)
    return _nc_cache[key]


def kernel_with_results(inputs, weight, bias, trace=False, **build_kw):
    x = np.ascontiguousarray(inputs, dtype=np.float32)
    w = np.ascontiguousarray(weight, dtype=np.float32)
    b = np.ascontiguousarray(bias, dtype=np.float32).reshape(C, 1)
    assert x.shape == (B, C, L), x.shape
    nc = _get_nc(**build_kw)
    in_maps = [
        {"x": x[i * BPC : (i + 1) * BPC], "w": w, "b": b} for i in range(NCORES)
    ]
    res = bass_utils.run_bass_kernel_spmd(
        nc, in_maps, core_ids=list(range(NCORES)), trace=trace
    )
    out = np.concatenate([r["y"] for r in res.results], axis=0)
    return out, res


def kernel(inputs, weight, bias):
    out, _ = kernel_with_results(inputs, weight, bias)
    return out
